# revision 1
# baseline (speedup 1.0000x reference)
"""BiLevelRoutingAttention (spiking, linear attention with window routing) on 8 TRN2 cores.

Sharding: 16 (t,b) pairs -> 2 per core, data-parallel. Host precomputes routing
(region sums -> top-k window indices) and passes x transposed; the device does
qkv projection (3-term f32r residual-split for fp32-grade accuracy), LIF spikes
(thresholds fused into PSUM evacuation via DVE is_ge / ACT sigmoid saturation),
per-window kv outer products (bf16, exact integer counts), top-k aggregation via
indirect-DMA row gathers + selector identity matmuls, linear attention (bf16),
and the output projection (f32r with residual-split weights for full precision),
producing the output transposed; host transposes back.
"""
import sys
sys.path.insert(0, '/opt/trn_rl_repo')

import numpy as np
import ml_dtypes

import concourse.bass as bass
import concourse.bacc as bacc
import concourse.mybir as mybir
from concourse.tile import TileContext
from concourse import bass_utils

F32 = mybir.dt.float32
F32R = mybir.dt.float32r
BF16 = mybir.dt.bfloat16
I32 = mybir.dt.int32
GE = mybir.AluOpType.is_ge
SIG = mybir.ActivationFunctionType.Sigmoid

T, B, L, C = 4, 4, 4096, 256
NW, TOPK, H, D = 8, 4, 4, 64
WIN = L // NW           # 512
NCORES = 8
NPAIR = 2               # (t,b) pairs per core
BIGS = 1.0e18           # sigmoid saturation scale

# toggles (fallbacks)
USE_DYN_MM = True       # dynamic register-offset rhs on aggregation matmuls
USE_ACT_SIG = True      # ACT sigmoid-saturation thresholds (else all DVE)

_EXEC_TIME_NS = None    # stashed for test harness


def _ensure_ntff_hook():
    """The agent image's antenv lacks axon_hooks; register the same hook
    trn_boot would have installed so trace=True can collect NTFF profiles."""
    import types
    try:
        import antenv.axon_hooks  # noqa: F401
        return True
    except ImportError:
        pass
    try:
        import antenv
        from trn_agent_boot.trn_boot import _ntff_profile_via_ctypes
        state = {"hook": _ntff_profile_via_ctypes('/opt/axon/libaxon_pjrt.so')}
        mod = types.ModuleType("antenv.axon_hooks")
        mod.get_axon_ntff_profile_hook = lambda: state["hook"]
        mod.set_axon_ntff_profile_hook = lambda h: state.__setitem__("hook", h)
        sys.modules["antenv.axon_hooks"] = mod
        antenv.axon_hooks = mod
        return True
    except Exception:
        return False


def f32r(ap):
    return ap.bitcast(F32R)


def _build_nc():
    nc = bacc.Bacc("TRN2", target_bir_lowering=False, debug=False,
                   num_devices=8)

    xt = nc.dram_tensor("xt", [NPAIR, C, L], F32, kind="ExternalInput")
    xtb = nc.dram_tensor("xtb", [NPAIR, C, L], F32, kind="ExternalInput")
    wqkv = nc.dram_tensor("wqkv", [C, 768], F32, kind="ExternalInput")
    wqv = nc.dram_tensor("wqv", [C, 768], F32, kind="ExternalInput")
    bkv = nc.dram_tensor("bkv", [1, 512], F32, kind="ExternalInput")
    thrq = nc.dram_tensor("thrq", [C, 1], F32, kind="ExternalInput")
    sigbq = nc.dram_tensor("sigbq", [C, 1], F32, kind="ExternalInput")
    wproj = nc.dram_tensor("wproj", [C, C], F32, kind="ExternalInput")
    wpv = nc.dram_tensor("wpv", [C, C], F32, kind="ExternalInput")
    thrp = nc.dram_tensor("thrp", [C, 1], F32, kind="ExternalInput")
    sigbp = nc.dram_tensor("sigbp", [C, 1], F32, kind="ExternalInput")
    idtop = nc.dram_tensor("idtop", [128, 128], BF16, kind="ExternalInput")
    idbot = nc.dram_tensor("idbot", [128, 128], BF16, kind="ExternalInput")
    onesrow = nc.dram_tensor("onesrow", [1, 128], F32, kind="ExternalInput")
    idxrow = nc.dram_tensor("idxrow", [NPAIR, 128, NW * TOPK], I32, kind="ExternalInput")
    out = nc.dram_tensor("out", [NPAIR, C, L], F32, kind="ExternalOutput")
    kvw_dram = nc.dram_tensor("kvw_scratch", [NPAIR * NW * 128, 128], BF16,
                              kind="Internal")

    with TileContext(nc) as tc:
        with (
            tc.tile_pool(name="const", bufs=1) as cpool,
            tc.tile_pool(name="xtp", bufs=1) as xtp,
            tc.tile_pool(name="big", bufs=1) as big,
            tc.tile_pool(name="small", bufs=2) as small,
            tc.tile_pool(name="psA", bufs=4, space="PSUM") as psA,
            tc.tile_pool(name="psB", bufs=1, space="PSUM") as psB,
            tc.tile_pool(name="psC", bufs=2, space="PSUM") as psC,
        ):
            # ---- constants / weights (once) ----
            w_sb = [cpool.tile([128, 768], F32R, tag="wq0", name="wq0"),
                    cpool.tile([128, 768], F32R, tag="wq1", name="wq1")]
            nc.gpsimd.dma_start(w_sb[0][:], wqkv[0:128, :])
            nc.gpsimd.dma_start(w_sb[1][:], wqkv[128:256, :])
            wv_sb = [cpool.tile([128, 768], F32R, tag="wv0", name="wv0"),
                     cpool.tile([128, 768], F32R, tag="wv1", name="wv1")]
            nc.gpsimd.dma_start(wv_sb[0][:], wqv[0:128, :])
            nc.gpsimd.dma_start(wv_sb[1][:], wqv[128:256, :])
            wp_sb = [cpool.tile([128, 256], F32R, tag="wp0", name="wp0"),
                     cpool.tile([128, 256], F32R, tag="wp1", name="wp1")]
            nc.gpsimd.dma_start(wp_sb[0][:], wproj[0:128, :])
            nc.gpsimd.dma_start(wp_sb[1][:], wproj[128:256, :])
            wpv_sb = [cpool.tile([128, 256], F32R, tag="wpv0", name="wpv0"),
                      cpool.tile([128, 256], F32R, tag="wpv1", name="wpv1")]
            nc.gpsimd.dma_start(wpv_sb[0][:], wpv[0:128, :])
            nc.gpsimd.dma_start(wpv_sb[1][:], wpv[128:256, :])
            bkv_sb = cpool.tile([1, 512], F32R, tag="bkv", name="bkv")
            nc.gpsimd.dma_start(bkv_sb[:], bkv[:])
            thrq_sb = cpool.tile([128, 2], F32, tag="thrq", name="thrq")
            nc.sync.dma_start(thrq_sb[:], thrq.rearrange("(a p) b -> p (a b)", p=128))
            sigbq_sb = cpool.tile([128, 2], F32, tag="sigbq", name="sigbq")
            nc.sync.dma_start(sigbq_sb[:], sigbq.rearrange("(a p) b -> p (a b)", p=128))
            thrp_sb = cpool.tile([128, 2], F32, tag="thrp", name="thrp")
            nc.sync.dma_start(thrp_sb[:], thrp.rearrange("(a p) b -> p (a b)", p=128))
            sigbp_sb = cpool.tile([128, 2], F32, tag="sigbp", name="sigbp")
            nc.sync.dma_start(sigbp_sb[:], sigbp.rearrange("(a p) b -> p (a b)", p=128))
            idt_sb = cpool.tile([128, 128], BF16, tag="idtop", name="idtop")
            nc.sync.dma_start(idt_sb[:], idtop[:])
            idb_sb = cpool.tile([128, 128], BF16, tag="idbot", name="idbot")
            nc.sync.dma_start(idb_sb[:], idbot[:])
            ones_sb = cpool.tile([1, 128], F32R, tag="ones", name="ones")
            nc.gpsimd.dma_start(ones_sb[:], onesrow[:])
            negbig_sb = cpool.tile([128, 1], F32, tag="negbig", name="negbig")
            nc.vector.memset(negbig_sb[:], -2.0 * BIGS)

            for p in range(NPAIR):
                # ---- load x^T for this pair ----
                xt_sb = [xtp.tile([128, L], F32R, tag="xt0", name="xt0"),
                         xtp.tile([128, L], F32R, tag="xt1", name="xt1")]
                xtb_sb = [xtp.tile([128, L], F32R, tag="xtb0", name="xtb0"),
                          xtp.tile([128, L], F32R, tag="xtb1", name="xtb1")]
                for q4 in range(4):
                    qs = slice(q4 * 1024, (q4 + 1) * 1024)
                    nc.sync.dma_start(xt_sb[0][:, qs], xt[p, 0:128, qs].bitcast(F32R))
                    nc.sync.dma_start(xt_sb[1][:, qs], xt[p, 128:256, qs].bitcast(F32R))
                    nc.sync.dma_start(xtb_sb[0][:, qs], xtb[p, 0:128, qs].bitcast(F32R))
                    nc.sync.dma_start(xtb_sb[1][:, qs], xtb[p, 128:256, qs].bitcast(F32R))
                idxrow_sb = small.tile([128, NW * TOPK], I32, tag="idxrow", name="idxrow")
                nc.sync.dma_start(idxrow_sb[:], idxrow[p, :, :])

                kv_sb = big.tile([128, 32 * 512], BF16, tag="kv", name="kv")
                qt_sb = [big.tile([128, L], BF16, tag="qt0", name="qt0"),
                         big.tile([128, L], BF16, tag="qt1", name="qt1")]

                # ---- k/v projection (3-term f32r), then per-window kvw ----
                kvw_sb = big.tile([128, 1024], BF16, tag="kvwsb", name="kvwsb")
                for m in range(32):
                    ps = psA.tile([128, 512], F32, tag="psA", name="psA")
                    msl = slice(m * 128, (m + 1) * 128)
                    nc.tensor.matmul(ps[:], xt_sb[0][:, msl],
                                     w_sb[0][:, 256:768], start=True, stop=False)
                    nc.tensor.matmul(ps[:], xt_sb[0][:, msl],
                                     wv_sb[0][:, 256:768], start=False, stop=False)
                    nc.tensor.matmul(ps[:], xt_sb[1][:, msl],
                                     w_sb[1][:, 256:768], start=False, stop=False)
                    nc.tensor.matmul(ps[:], xt_sb[1][:, msl],
                                     wv_sb[1][:, 256:768], start=False, stop=False)
                    nc.tensor.matmul(ps[:], xtb_sb[0][:, msl],
                                     w_sb[0][:, 256:768], start=False, stop=False)
                    nc.tensor.matmul(ps[:], xtb_sb[1][:, msl],
                                     w_sb[1][:, 256:768], start=False, stop=False)
                    nc.tensor.matmul(ps[:], ones_sb[:], bkv_sb[:],
                                     start=False, stop=True)
                    dst = kv_sb[:, m * 512:(m + 1) * 512]
                    if USE_ACT_SIG and m % 2 == 1:
                        nc.scalar.activation(dst, ps[:], SIG,
                                             bias=negbig_sb[:], scale=BIGS)
                    else:
                        nc.vector.tensor_scalar(dst, ps[:], 2.0, None, GE)
                # phase B batched: one [128,128] bf16 matmul per (j, head-pair,
                # chunk) computes both heads' kvw diagonal blocks at once (off-
                # diagonal cross-head blocks are computed and discarded). Two
                # 4-window rounds keep the PSUM footprint at 2 banks.
                for rnd in range(2):
                    kvwf = psB.tile([128, 1024], F32, tag="kvw", name="kvwf")
                    for jl in range(4):
                        j = rnd * 4 + jl
                        for hp in range(2):
                            blk = (2 * jl + hp) * 128
                            for c in range(4):
                                col = (4 * j + c) * 512
                                nc.tensor.matmul(
                                    kvwf[:, blk:blk + 128],
                                    kv_sb[:, col + hp * 128: col + hp * 128 + 128],
                                    kv_sb[:, col + 256 + hp * 128: col + 256 + hp * 128 + 128],
                                    start=(jl % 2 == 0 and hp == 0 and c == 0),
                                    stop=(jl % 2 == 1 and hp == 1 and c == 3),
                                    skip_group_check=True)
                    # extract diagonal sub-blocks: kvw_sb[s*64+d, j*128+hp*64+e]
                    #   <- kvwf[s*64+d, (2*jl+hp)*128 + s*64 + e]
                    for s in range(2):
                        srows = slice(s * 64, (s + 1) * 64)
                        srcap = kvwf[srows, :].rearrange(
                            "q (b e) -> q b e", e=128)[:, :, s * 64:s * 64 + 64]
                        dstap = kvw_sb[srows, rnd * 512:(rnd + 1) * 512].rearrange(
                            "q (b e) -> q b e", e=64)
                        if s == 0:
                            nc.vector.tensor_copy(dstap, srcap)
                        else:
                            nc.scalar.copy(dstap, srcap)
                    for jl in range(4):
                        j = rnd * 4 + jl
                        nc.sync.dma_start(
                            kvw_dram[(p * NW + j) * 128:(p * NW + j + 1) * 128, :],
                            kvw_sb[:, j * 128:(j + 1) * 128])

                # ---- indirect gathers issued now; q^T matmuls below overlap ----
                gath = big.tile([128, NW * TOPK * 128], BF16, tag="gath", name="gath")
                for n in range(NW):
                    for i in range(TOPK):
                        m = n * TOPK + i
                        nc.gpsimd.indirect_dma_start(
                            out=gath[:, m * 128:(m + 1) * 128],
                            out_offset=None,
                            in_=kvw_dram[:],
                            in_offset=bass.IndirectOffsetOnAxis(
                                ap=idxrow_sb[:, m:m + 1], axis=0),
                        )

                # ---- q^T projection (3-term f32r), fills PE during gathers ----
                nevac = 0
                for g in range(8):
                    for dq in range(2):
                        ps = psA.tile([128, 512], F32, tag="psA", name="psA")
                        gsl = slice(g * 512, (g + 1) * 512)
                        dsl = slice(dq * 128, (dq + 1) * 128)
                        nc.tensor.matmul(ps[:], w_sb[0][:, dsl], xt_sb[0][:, gsl],
                                         start=True, stop=False)
                        nc.tensor.matmul(ps[:], w_sb[0][:, dsl], xtb_sb[0][:, gsl],
                                         start=False, stop=False)
                        nc.tensor.matmul(ps[:], w_sb[1][:, dsl], xt_sb[1][:, gsl],
                                         start=False, stop=False)
                        nc.tensor.matmul(ps[:], w_sb[1][:, dsl], xtb_sb[1][:, gsl],
                                         start=False, stop=False)
                        nc.tensor.matmul(ps[:], wv_sb[0][:, dsl], xt_sb[0][:, gsl],
                                         start=False, stop=False)
                        nc.tensor.matmul(ps[:], wv_sb[1][:, dsl], xt_sb[1][:, gsl],
                                         start=False, stop=True)
                        dst = qt_sb[dq][:, g * 512:(g + 1) * 512]
                        if USE_ACT_SIG and nevac % 2 == 0:
                            nc.scalar.activation(dst, ps[:], SIG,
                                                 bias=sigbq_sb[:, dq:dq + 1], scale=BIGS)
                        else:
                            nc.vector.tensor_scalar(dst, ps[:], thrq_sb[:, dq:dq + 1],
                                                    None, GE)
                        nevac += 1

                # ---- aggregation into block-diagonal kv_g (two 2-bank halves) ----
                # PSUM note: start=True clears the whole bank's has_written bits,
                # so only the FIRST matmul touching each 512-col bank may set it.
                kvg_sb = big.tile([128, 2048], BF16, tag="kvgsb", name="kvgsb")
                for half in range(2):
                    kvg_ps = psB.tile([128, 1024], F32, tag="kvw", name="kvg")
                    for nl in range(4):
                        n = half * 4 + nl
                        base = kvg_ps[:, nl * 256:(nl + 1) * 256]
                        top = base.rearrange("q (hp e) -> q hp e", hp=2)[:, :, 0:64]
                        bot = base.rearrange("q (hp e) -> q hp e", hp=2)[:, :, 64:128]
                        for i in range(TOPK):
                            m = n * TOPK + i
                            rhs = gath[:, m * 128:(m + 1) * 128]
                            nc.tensor.matmul(top, idt_sb[:], rhs,
                                             start=(nl % 2 == 0 and i == 0),
                                             stop=False, skip_group_check=True)
                            nc.tensor.matmul(bot, idb_sb[:], rhs,
                                             start=False,
                                             stop=(nl % 2 == 1 and i == TOPK - 1),
                                             skip_group_check=True)
                    hdst = kvg_sb[:, half * 1024:(half + 1) * 1024]
                    if half == 0:
                        nc.vector.tensor_copy(hdst, kvg_ps[:])
                    else:
                        nc.scalar.copy(hdst, kvg_ps[:])

                # ---- phase C: out^T[e + 64h, w] = kv_g_h^T-as-lhsT @ q_h^T ----
                outT_sb = [big.tile([128, L], F32R, tag="ot0", name="ot0"),
                           big.tile([128, L], F32R, tag="ot1", name="ot1")]
                for n in range(NW):
                    for hp in range(2):
                        ps = psC.tile([128, 512], F32, tag="psCt", name="psCt")
                        nc.tensor.matmul(
                            ps[:],
                            kvg_sb[:, n * 256 + hp * 128: n * 256 + hp * 128 + 128],
                            qt_sb[hp][:, n * 512:(n + 1) * 512],
                            start=True, stop=True)
                        dst = outT_sb[hp][:, n * 512:(n + 1) * 512]
                        nc.vector.tensor_copy(dst, ps[:])

                # ---- phase D: fin^T = (W_proj^T-chunks @ out^T >= thr) ----
                for g in range(8):
                    fin_sb = small.tile([128, 1024], F32, tag="fin", name="fin")
                    for ct in range(2):
                        ps = psC.tile([128, 512], F32, tag="psCt", name="psD")
                        csl = slice(ct * 128, (ct + 1) * 128)
                        gsl = slice(g * 512, (g + 1) * 512)
                        nc.tensor.matmul(ps[:], wp_sb[0][:, csl], outT_sb[0][:, gsl],
                                         start=True, stop=False)
                        nc.tensor.matmul(ps[:], wp_sb[1][:, csl], outT_sb[1][:, gsl],
                                         start=False, stop=False)
                        nc.tensor.matmul(ps[:], wpv_sb[0][:, csl], outT_sb[0][:, gsl],
                                         start=False, stop=False)
                        nc.tensor.matmul(ps[:], wpv_sb[1][:, csl], outT_sb[1][:, gsl],
                                         start=False, stop=True)
                        dst = fin_sb[:, ct * 512:(ct + 1) * 512]
                        nc.scalar.activation(dst, ps[:], SIG,
                                             bias=sigbp_sb[:, ct:ct + 1], scale=BIGS)
                    nc.sync.dma_start(out[p, 0:128, g * 512:(g + 1) * 512],
                                      fin_sb[:, 0:512])
                    nc.sync.dma_start(out[p, 128:256, g * 512:(g + 1) * 512],
                                      fin_sb[:, 512:1024])

    nc.compile()
    return nc


_NC = None


def _f32r_round(a):
    """Round fp32 to the f32r grid (12-bit significand, round-to-nearest)."""
    u = np.ascontiguousarray(a, dtype=np.float32).view(np.uint32)
    u = (u + np.uint32(1 << 11)) & np.uint32(0xFFFFF000)
    return u.view(np.float32)


def kernel(x, W_qkv, b_qkv, W_proj, b_proj):
    global _NC, _EXEC_TIME_NS
    x = np.asarray(x, dtype=np.float32)
    W_qkv = np.asarray(W_qkv, dtype=np.float32)
    b_qkv = np.asarray(b_qkv, dtype=np.float32)
    W_proj = np.asarray(W_proj, dtype=np.float32)
    b_proj = np.asarray(b_proj, dtype=np.float32)

    # ---- host routing: region sums -> attn -> top-k window indices ----
    region = x.sum(axis=0).reshape(B, NW, WIN, C).sum(axis=2)        # [B,NW,C]
    attn_r = np.einsum('bnc,bmc->bnm', region, region)
    idx = np.argsort(-attn_r, axis=-1, kind='stable')[:, :, :TOPK]   # [B,NW,TOPK]

    # ---- common (replicated) inputs ----
    wq_u = _f32r_round(W_qkv)
    wp_u = _f32r_round(W_proj)
    common = {
        "wqkv": wq_u,
        "wqv": np.ascontiguousarray(W_qkv - wq_u),
        "wpv": np.ascontiguousarray(W_proj - wp_u),
        "bkv": np.ascontiguousarray(b_qkv[None, 256:768]),
        "thrq": np.ascontiguousarray(2.0 - b_qkv[0:256, None]),
        "sigbq": np.ascontiguousarray(-BIGS * (2.0 - b_qkv[0:256, None])).astype(np.float32),
        "wproj": wp_u,
        "thrp": np.ascontiguousarray(2.0 - b_proj[:, None]),
        "sigbp": np.ascontiguousarray(-BIGS * (2.0 - b_proj[:, None])).astype(np.float32),
        "idtop": np.diag(np.r_[np.ones(64), np.zeros(64)]).astype(ml_dtypes.bfloat16),
        "idbot": np.diag(np.r_[np.zeros(64), np.ones(64)]).astype(ml_dtypes.bfloat16),
        "onesrow": np.ones((1, 128), dtype=np.float32),
    }

    in_maps = []
    pairs = [(t, b) for t in range(T) for b in range(B)]
    for core in range(NCORES):
        mine = pairs[core * NPAIR:(core + 1) * NPAIR]
        xt_full = np.stack([np.ascontiguousarray(x[t, b].T) for (t, b) in mine])
        xt = _f32r_round(xt_full)
        xtb_arr = xt_full - xt
        rows = []
        for k, (t, b) in enumerate(mine):
            # idxrow[q, i*NW + n] = row of kvw_dram for pair k, window idx[b,n,i],
            # partition q
            r = np.empty((128, NW * TOPK), dtype=np.int32)
            for n in range(NW):
                for i in range(TOPK):
                    r[:, n * TOPK + i] = (k * NW + idx[b, n, i]) * 128 + np.arange(128)
            rows.append(r)
        m = dict(common)
        m["xt"] = xt
        m["xtb"] = xtb_arr
        m["idxrow"] = np.stack(rows)
        in_maps.append(m)

    if _NC is None:
        _NC = _build_nc()

    traceable = _ensure_ntff_hook()
    try:
        res = bass_utils.run_bass_kernel_spmd(_NC, in_maps,
                                              core_ids=list(range(NCORES)),
                                              trace=traceable)
    except Exception:
        if not traceable:
            raise
        res = bass_utils.run_bass_kernel_spmd(_NC, in_maps,
                                              core_ids=list(range(NCORES)),
                                              trace=False)
    _EXEC_TIME_NS = res.exec_time_ns

    full = np.empty((T, B, L, C), dtype=np.float32)
    for core in range(NCORES):
        mine = pairs[core * NPAIR:(core + 1) * NPAIR]
        o = res.results[core]["out"]                                  # [NPAIR, C, L]
        for k, (t, b) in enumerate(mine):
            full[t, b] = o[k].T
    return full



# revision 6
# speedup vs baseline: 1.1769x; 1.1769x over previous
"""BiLevelRoutingAttention (spiking, linear attention with window routing) on 8 TRN2 cores.

Sharding: 16 (t,b) pairs -> 2 per core, data-parallel. Host precomputes routing
(region sums -> top-k window indices -> 0/1 selection masks); the device does
qkv projection (3-term f32r residual-split for fp32-grade accuracy, thresholds
fused into the PSUM evacuation), per-window kv outer products (fp8 DoubleRow on
the binary spikes), mask-weighted window aggregation on the vector/pool engines
(no DRAM round-trip), linear attention (bf16), and the output projection
(2-term: f32r main + fp8-DoubleRow residual at a 2^9 PSUM scale), producing
binary spikes in fp8 that the host converts back to f32 and transposes.
"""
import sys
sys.path.insert(0, '/opt/trn_rl_repo')

import numpy as np
import ml_dtypes

import concourse.bass as bass
import concourse.bacc as bacc
import concourse.mybir as mybir
from concourse.tile import TileContext
from concourse import bass_utils

F32 = mybir.dt.float32
F32R = mybir.dt.float32r
BF16 = mybir.dt.bfloat16
F8 = mybir.dt.float8e4
GE = mybir.AluOpType.is_ge
MULT = mybir.AluOpType.mult
ADD = mybir.AluOpType.add
SIG = mybir.ActivationFunctionType.Sigmoid
COPYF = mybir.ActivationFunctionType.Copy
DRMODE = mybir.MatmulPerfMode.DoubleRow

T, B, L, C = 4, 4, 4096, 256
NW, TOPK, H, D = 8, 4, 4, 64
WIN = L // NW           # 512
NCORES = 8
NPAIR = 2               # (t,b) pairs per core
BIGS = 1.0e18           # sigmoid saturation scale
PS = 512.0              # proj main-term PSUM scale (= 2^-5 * 2^14)

_EXEC_TIME_NS = None    # stashed for test harness


def _ensure_ntff_hook():
    """The agent image's antenv lacks axon_hooks; register the same hook
    trn_boot would have installed so trace=True can collect NTFF profiles."""
    import types
    try:
        import antenv.axon_hooks  # noqa: F401
        return True
    except ImportError:
        pass
    try:
        import antenv
        from trn_agent_boot.trn_boot import _ntff_profile_via_ctypes
        state = {"hook": _ntff_profile_via_ctypes('/opt/axon/libaxon_pjrt.so')}
        mod = types.ModuleType("antenv.axon_hooks")
        mod.get_axon_ntff_profile_hook = lambda: state["hook"]
        mod.set_axon_ntff_profile_hook = lambda h: state.__setitem__("hook", h)
        sys.modules["antenv.axon_hooks"] = mod
        antenv.axon_hooks = mod
        return True
    except Exception:
        return False


def _build_nc():
    nc = bacc.Bacc("TRN2", target_bir_lowering=False, debug=False,
                   num_devices=8)

    xt = nc.dram_tensor("xt", [NPAIR, C, L], F32, kind="ExternalInput")
    xtb = nc.dram_tensor("xtb", [NPAIR, C, L], F32, kind="ExternalInput")
    wqkv = nc.dram_tensor("wqkv", [C, 768], F32, kind="ExternalInput")
    wqv = nc.dram_tensor("wqv", [C, 768], F32, kind="ExternalInput")
    thrkv = nc.dram_tensor("thrkv", [128, 512], F32, kind="ExternalInput")
    thrq = nc.dram_tensor("thrq", [C, 1], F32, kind="ExternalInput")
    sigbq = nc.dram_tensor("sigbq", [C, 1], F32, kind="ExternalInput")
    wps = nc.dram_tensor("wps", [C, C], F32, kind="ExternalInput")
    wpl8 = nc.dram_tensor("wpl8", [128, 512], F8, kind="ExternalInput")
    thrp = nc.dram_tensor("thrp", [C, 1], F32, kind="ExternalInput")
    sigbp = nc.dram_tensor("sigbp", [C, 1], F32, kind="ExternalInput")
    maskr = nc.dram_tensor("maskr", [NPAIR, 128, NW * NW], F32,
                           kind="ExternalInput")
    out = nc.dram_tensor("out", [NPAIR, C, L], F8, kind="ExternalOutput")

    with TileContext(nc) as tc:
        with (
            tc.tile_pool(name="const", bufs=1) as cpool,
            tc.tile_pool(name="xtp", bufs=1) as xtp,
            tc.tile_pool(name="big", bufs=1) as big,
            tc.tile_pool(name="dbl", bufs=2) as dbl,
            tc.tile_pool(name="fin", bufs=4) as finp,
            tc.tile_pool(name="psA", bufs=4, space="PSUM") as psA,
            tc.tile_pool(name="psB", bufs=2, space="PSUM") as psB,
        ):
            # ---- constants / weights (once) ----
            w_sb = [cpool.tile([128, 768], F32R, tag="wq0", name="wq0"),
                    cpool.tile([128, 768], F32R, tag="wq1", name="wq1")]
            nc.gpsimd.dma_start(w_sb[0][:], wqkv[0:128, :])
            nc.gpsimd.dma_start(w_sb[1][:], wqkv[128:256, :])
            wv_sb = [cpool.tile([128, 768], F32R, tag="wv0", name="wv0"),
                     cpool.tile([128, 768], F32R, tag="wv1", name="wv1")]
            nc.gpsimd.dma_start(wv_sb[0][:], wqv[0:128, :])
            nc.gpsimd.dma_start(wv_sb[1][:], wqv[128:256, :])
            wp_sb = [cpool.tile([128, 256], F32R, tag="wp0", name="wp0"),
                     cpool.tile([128, 256], F32R, tag="wp1", name="wp1")]
            nc.gpsimd.dma_start(wp_sb[0][:], wps[0:128, :])
            nc.gpsimd.dma_start(wp_sb[1][:], wps[128:256, :])
            wpl8_sb = cpool.tile([128, 512], F8, tag="wpl8", name="wpl8")
            nc.gpsimd.dma_start(wpl8_sb[:], wpl8[:])
            thrkv_sb = cpool.tile([128, 512], F32, tag="thrkv", name="thrkv")
            nc.gpsimd.dma_start(thrkv_sb[:], thrkv[:])
            thrq_sb = cpool.tile([128, 2], F32, tag="thrq", name="thrq")
            nc.sync.dma_start(thrq_sb[:], thrq.rearrange("(a p) b -> p (a b)", p=128))
            sigbq_sb = cpool.tile([128, 2], F32, tag="sigbq", name="sigbq")
            nc.sync.dma_start(sigbq_sb[:], sigbq.rearrange("(a p) b -> p (a b)", p=128))
            thrp_sb = cpool.tile([128, 2], F32, tag="thrp", name="thrp")
            nc.sync.dma_start(thrp_sb[:], thrp.rearrange("(a p) b -> p (a b)", p=128))
            sigbp_sb = cpool.tile([128, 2], F32, tag="sigbp", name="sigbp")
            nc.sync.dma_start(sigbp_sb[:], sigbp.rearrange("(a p) b -> p (a b)", p=128))
            mask_sb = [cpool.tile([128, NW * NW], F32, tag="mask0", name="mask0"),
                       cpool.tile([128, NW * NW], F32, tag="mask1", name="mask1")]
            nc.gpsimd.dma_start(mask_sb[0][:], maskr[0, :, :])
            nc.gpsimd.dma_start(mask_sb[1][:], maskr[1, :, :])

            # ---- per-pair x quarter tiles (slots shared across pairs) ----
            def load_x(p):
                tiles = []
                for q in range(4):
                    qs = slice(q * 1024, (q + 1) * 1024)
                    tq = {}
                    for nm, src in (("xt0", xt[p, 0:128, qs]),
                                    ("xt1", xt[p, 128:256, qs]),
                                    ("xb0", xtb[p, 0:128, qs]),
                                    ("xb1", xtb[p, 128:256, qs])):
                        t = xtp.tile([128, 1024], F32R, tag=f"{nm}q{q}",
                                     name=f"{nm}q{q}")
                        nc.sync.dma_start(t[:], src.bitcast(F32R))
                        tq[nm] = t
                    tiles.append(tq)
                return tiles

        # state kept per pair between phases
            xq = [None, None]
            kv_sb = [None, None]
            kvw_sb = [None, None]
            kvg_sb = [None, None]
            qt_sb = [None, None]
            outT = [None, None]
            out8 = [None, None]

            def phase_AB(p):
                kv_sb[p] = big.tile([128, 32 * 512], F8, tag="kv", name="kv")
                kvw_sb[p] = dbl.tile([128, NW * 256], BF16, tag="kvw", name="kvw")
                nc.vector.memset(kvw_sb[p][:], 0.0)
                xqp = xq[p]
                for m in range(32):
                    q, mo = m // 8, (m % 8) * 128
                    msl = slice(mo, mo + 128)
                    ps = psA.tile([128, 512], F32, tag="psA", name="psA")
                    nc.tensor.matmul(ps[:], xqp[q]["xt0"][:, msl],
                                     w_sb[0][:, 256:768], start=True, stop=False)
                    nc.tensor.matmul(ps[:], xqp[q]["xt0"][:, msl],
                                     wv_sb[0][:, 256:768], start=False, stop=False)
                    nc.tensor.matmul(ps[:], xqp[q]["xt1"][:, msl],
                                     w_sb[1][:, 256:768], start=False, stop=False)
                    nc.tensor.matmul(ps[:], xqp[q]["xt1"][:, msl],
                                     wv_sb[1][:, 256:768], start=False, stop=False)
                    nc.tensor.matmul(ps[:], xqp[q]["xb0"][:, msl],
                                     w_sb[0][:, 256:768], start=False, stop=False)
                    nc.tensor.matmul(ps[:], xqp[q]["xb1"][:, msl],
                                     w_sb[1][:, 256:768], start=False, stop=True)
                    nc.vector.tensor_tensor(kv_sb[p][:, m * 512:(m + 1) * 512],
                                            ps[:], thrkv_sb[:], op=GE)
                    # B round after every 8 A-tiles: windows 2r, 2r+1
                    if m % 8 == 7:
                        r = m // 8
                        kvwf = psB.tile([128, 512], F32, tag="kvwf", name="kvwf")
                        for wl in range(2):
                            j = 2 * r + wl
                            base = j * 2048
                            kvv = kv_sb[p][:, base:base + 1024].rearrange(
                                "p (two x) -> p two x", two=2)
                            kvv2 = kv_sb[p][:, base + 1024:base + 2048].rearrange(
                                "p (two x) -> p two x", two=2)
                            for hp in range(2):
                                ksl = slice(hp * 128, hp * 128 + 128)
                                vsl = slice(256 + hp * 128, 256 + hp * 128 + 128)
                                blk = (2 * wl + hp) * 128
                                nc.tensor.matmul(
                                    kvwf[:, blk:blk + 128],
                                    kvv[:, :, ksl], kvv[:, :, vsl],
                                    start=(wl == 0 and hp == 0), stop=False,
                                    perf_mode=DRMODE, skip_group_check=True)
                                nc.tensor.matmul(
                                    kvwf[:, blk:blk + 128],
                                    kvv2[:, :, ksl], kvv2[:, :, vsl],
                                    start=False, stop=(wl == 1 and hp == 1),
                                    perf_mode=DRMODE, skip_group_check=True)
                        # extract diag [64,64] blocks -> kvw_sb (packed block-diag)
                        for s in range(2):
                            srows = slice(s * 64, (s + 1) * 64)
                            srcap = kvwf[srows, :].rearrange(
                                "q (b e) -> q b e", e=128)[:, :, s * 64:s * 64 + 64]
                            dstap = kvw_sb[p][srows, r * 512:(r + 1) * 512].rearrange(
                                "q (b e) -> q b e", e=128)[:, :, s * 64:s * 64 + 64]
                            if s == 0:
                                nc.vector.tensor_copy(dstap, srcap)
                            else:
                                nc.scalar.copy(dstap, srcap)

            def phase_qT(p):
                qt_sb[p] = [dbl.tile([128, L], BF16, tag="qt0", name="qt0"),
                            dbl.tile([128, L], BF16, tag="qt1", name="qt1")]
                xqp = xq[p]
                for g in range(8):
                    q, go = g // 2, (g % 2) * 512
                    gsl = slice(go, go + 512)
                    for dq in range(2):
                        ps = psA.tile([128, 512], F32, tag="psA", name="psA")
                        dsl = slice(dq * 128, (dq + 1) * 128)
                        nc.tensor.matmul(ps[:], w_sb[0][:, dsl], xqp[q]["xt0"][:, gsl],
                                         start=True, stop=False)
                        nc.tensor.matmul(ps[:], w_sb[0][:, dsl], xqp[q]["xb0"][:, gsl],
                                         start=False, stop=False)
                        nc.tensor.matmul(ps[:], w_sb[1][:, dsl], xqp[q]["xt1"][:, gsl],
                                         start=False, stop=False)
                        nc.tensor.matmul(ps[:], w_sb[1][:, dsl], xqp[q]["xb1"][:, gsl],
                                         start=False, stop=False)
                        nc.tensor.matmul(ps[:], wv_sb[0][:, dsl], xqp[q]["xt0"][:, gsl],
                                         start=False, stop=False)
                        nc.tensor.matmul(ps[:], wv_sb[1][:, dsl], xqp[q]["xt1"][:, gsl],
                                         start=False, stop=True)
                        dst = qt_sb[p][dq][:, g * 512:(g + 1) * 512]
                        if dq == 0:
                            nc.scalar.activation(dst, ps[:], SIG,
                                                 bias=sigbq_sb[:, 0:1], scale=BIGS)
                        else:
                            nc.vector.tensor_scalar(dst, ps[:], thrq_sb[:, 1:2],
                                                    None, GE)

            def phase_agg(p):
                kvg_sb[p] = dbl.tile([128, NW * 256], BF16, tag="kvg", name="kvg")
                for n in range(NW):
                    ksl = slice(n * 256, (n + 1) * 256)
                    nc.gpsimd.tensor_scalar(kvg_sb[p][:, ksl],
                                            kvw_sb[p][:, 0:256],
                                            mask_sb[p][:, n * 8:n * 8 + 1],
                                            None, MULT)
                    for w in range(1, NW):
                        nc.vector.scalar_tensor_tensor(
                            kvg_sb[p][:, ksl], kvw_sb[p][:, w * 256:(w + 1) * 256],
                            mask_sb[p][:, n * 8 + w:n * 8 + w + 1],
                            kvg_sb[p][:, ksl], op0=MULT, op1=ADD)

            def phase_C(p):
                outT[p] = [big.tile([128, L], F32R, tag="ot0", name="ot0"),
                           big.tile([128, L], F32R, tag="ot1", name="ot1")]
                out8[p] = big.tile([128, 2 * L], F8, tag="o8", name="o8")
                for n in range(NW):
                    for dq in range(2):
                        ps = psA.tile([128, 512], F32, tag="psA", name="psC")
                        nc.tensor.matmul(
                            ps[:],
                            kvg_sb[p][:, n * 256 + dq * 128: n * 256 + dq * 128 + 128],
                            qt_sb[p][dq][:, n * 512:(n + 1) * 512],
                            start=True, stop=True)
                        dst = outT[p][dq][:, n * 512:(n + 1) * 512]
                        d8 = out8[p][:, dq * L + n * 512: dq * L + (n + 1) * 512]
                        if (n + dq) % 2 == 0:
                            nc.vector.tensor_copy(dst, ps[:])
                            nc.scalar.activation(d8, ps[:], COPYF, scale=0.03125)
                        else:
                            nc.scalar.copy(dst, ps[:])
                            nc.vector.tensor_scalar(d8, ps[:], 0.03125, None, MULT)

            def phase_D(p):
                w8v = wpl8_sb[:].rearrange("p (two oc) -> p two oc", two=2)
                o8v = out8[p][:].rearrange("p (two x) -> p two x", two=2)
                for g in range(8):
                    for ct in range(2):
                        ps = psA.tile([128, 512], F32, tag="psA", name="psD")
                        csl = slice(ct * 128, (ct + 1) * 128)
                        gsl = slice(g * 512, (g + 1) * 512)
                        nc.tensor.matmul(ps[:], wp_sb[0][:, csl], outT[p][0][:, gsl],
                                         start=True, stop=False,
                                         skip_group_check=True)
                        nc.tensor.matmul(ps[:], wp_sb[1][:, csl], outT[p][1][:, gsl],
                                         start=False, stop=False,
                                         skip_group_check=True)
                        for sub in range(4):
                            nc.tensor.matmul(
                                ps[:, sub * 128:(sub + 1) * 128],
                                w8v[:, :, csl],
                                o8v[:, :, g * 512 + sub * 128: g * 512 + (sub + 1) * 128],
                                start=False, stop=(sub == 3),
                                perf_mode=DRMODE, skip_group_check=True)
                        fin = finp.tile([128, 512], F8, tag="fin", name="fin")
                        if ct == 0:
                            nc.scalar.activation(fin[:], ps[:], SIG,
                                                 bias=sigbp_sb[:, 0:1], scale=BIGS)
                        else:
                            nc.vector.tensor_scalar(fin[:], ps[:], thrp_sb[:, 1:2],
                                                    None, GE)
                        nc.sync.dma_start(
                            out[p, ct * 128:(ct + 1) * 128, g * 512:(g + 1) * 512],
                            fin[:])

            # ---- zippered emission ----
            xq[0] = load_x(0)
            xq[1] = load_x(1)
            phase_AB(0)
            phase_qT(0)
            phase_agg(0)
            phase_AB(1)
            phase_C(0)
            phase_D(0)
            phase_agg(1)
            phase_qT(1)
            phase_C(1)
            phase_D(1)

    nc.compile()
    return nc


_NC = None


def _f32r_round(a):
    """Round fp32 to the f32r grid (12-bit significand, round-to-nearest)."""
    u = np.ascontiguousarray(a, dtype=np.float32).view(np.uint32)
    u = (u + np.uint32(1 << 11)) & np.uint32(0xFFFFF000)
    return u.view(np.float32)


def kernel(x, W_qkv, b_qkv, W_proj, b_proj):
    global _NC, _EXEC_TIME_NS
    x = np.asarray(x, dtype=np.float32)
    W_qkv = np.asarray(W_qkv, dtype=np.float32)
    b_qkv = np.asarray(b_qkv, dtype=np.float32)
    W_proj = np.asarray(W_proj, dtype=np.float32)
    b_proj = np.asarray(b_proj, dtype=np.float32)

    # ---- host routing: region sums -> attn -> top-k -> 0/1 masks ----
    region = x.sum(axis=0).reshape(B, NW, WIN, C).sum(axis=2)        # [B,NW,C]
    attn_r = np.einsum('bnc,bmc->bnm', region, region)
    idx = np.argsort(-attn_r, axis=-1, kind='stable')[:, :, :TOPK]   # [B,NW,TOPK]
    masks = np.zeros((B, NW, NW), np.float32)
    for b in range(B):
        for n in range(NW):
            masks[b, n, idx[b, n]] = 1.0

    # ---- common (replicated) inputs ----
    wq_u = _f32r_round(W_qkv)
    wp_u = _f32r_round(W_proj)
    wpl = (W_proj - wp_u) * 16384.0
    wpl8_np = np.empty((128, 512), dtype=ml_dtypes.float8_e4m3)
    wpl8_np[:, 0:256] = wpl[0:128, :].astype(ml_dtypes.float8_e4m3)
    wpl8_np[:, 256:512] = wpl[128:256, :].astype(ml_dtypes.float8_e4m3)
    common = {
        "wqkv": wq_u,
        "wqv": np.ascontiguousarray(W_qkv - wq_u),
        "thrkv": np.ascontiguousarray(
            np.broadcast_to(2.0 - b_qkv[None, 256:768], (128, 512))),
        "thrq": np.ascontiguousarray(2.0 - b_qkv[0:256, None]),
        "sigbq": np.ascontiguousarray(
            -BIGS * (2.0 - b_qkv[0:256, None])).astype(np.float32),
        "wps": np.ascontiguousarray(wp_u * PS),
        "wpl8": wpl8_np,
        "thrp": np.ascontiguousarray(PS * (2.0 - b_proj[:, None])),
        "sigbp": np.ascontiguousarray(
            -BIGS * PS * (2.0 - b_proj[:, None])).astype(np.float32),
    }

    in_maps = []
    pairs = [(t, b) for t in range(T) for b in range(B)]
    for core in range(NCORES):
        mine = pairs[core * NPAIR:(core + 1) * NPAIR]
        xt_full = np.stack([np.ascontiguousarray(x[t, b].T) for (t, b) in mine])
        xt = _f32r_round(xt_full)
        m = dict(common)
        m["xt"] = xt
        m["xtb"] = xt_full - xt
        m["maskr"] = np.stack([
            np.broadcast_to(masks[b].reshape(1, NW * NW), (128, NW * NW))
            for (t, b) in mine]).astype(np.float32)
        in_maps.append(m)

    if _NC is None:
        _NC = _build_nc()

    traceable = _ensure_ntff_hook()
    try:
        res = bass_utils.run_bass_kernel_spmd(_NC, in_maps,
                                              core_ids=list(range(NCORES)),
                                              trace=traceable)
    except Exception:
        if not traceable:
            raise
        res = bass_utils.run_bass_kernel_spmd(_NC, in_maps,
                                              core_ids=list(range(NCORES)),
                                              trace=False)
    _EXEC_TIME_NS = res.exec_time_ns

    full = np.empty((T, B, L, C), dtype=np.float32)
    for core in range(NCORES):
        mine = pairs[core * NPAIR:(core + 1) * NPAIR]
        o = np.asarray(res.results[core]["out"]).astype(np.float32)  # [NPAIR,C,L]
        for k, (t, b) in enumerate(mine):
            full[t, b] = o[k].T
    return full


# revision 8
# speedup vs baseline: 1.4136x; 1.2011x over previous
"""BiLevelRoutingAttention (spiking, linear attention with window routing) on 8 TRN2 cores.

Sharding: 16 (t,b) pairs -> 2 per core, data-parallel. Host precomputes routing
(region sums -> top-k window indices -> gather row indices); the device does
qkv projection (3-term f32r residual-split for fp32-grade accuracy, thresholds
fused into paired-PSUM-bank evacuations), per-window kv outer products (fp8
DoubleRow on the binary spikes), top-k aggregation via indirect-DMA row gathers
with CCE-add accumulation (runs on the otherwise-idle gpsimd DMA queue),
linear attention (bf16), and the output projection (2-term: f32r main + fp8
DoubleRow residual at a 2^9 PSUM scale), producing binary spikes in fp8 that
the host converts back to f32 and transposes. The two pairs per core are
software-pipelined so the tensor engine never drains.
"""
import sys
sys.path.insert(0, '/opt/trn_rl_repo')

import numpy as np
import ml_dtypes

import concourse.bass as bass
import concourse.bacc as bacc
import concourse.mybir as mybir
from concourse.tile import TileContext
from concourse import bass_utils

F32 = mybir.dt.float32
F32R = mybir.dt.float32r
BF16 = mybir.dt.bfloat16
F8 = mybir.dt.float8e4
I32 = mybir.dt.int32
GE = mybir.AluOpType.is_ge
MULT = mybir.AluOpType.mult
ADD = mybir.AluOpType.add
SIG = mybir.ActivationFunctionType.Sigmoid
COPYF = mybir.ActivationFunctionType.Copy
DRMODE = mybir.MatmulPerfMode.DoubleRow

T, B, L, C = 4, 4, 4096, 256
NW, TOPK, H, D = 8, 4, 4, 64
WIN = L // NW           # 512
NCORES = 8
NPAIR = 2               # (t,b) pairs per core
BIGS = 1.0e18           # sigmoid saturation scale
PS = 512.0              # proj main-term PSUM scale (= 2^-5 * 2^14)

_EXEC_TIME_NS = None    # stashed for test harness


def _ensure_ntff_hook():
    """The agent image's antenv lacks axon_hooks; register the same hook
    trn_boot would have installed so trace=True can collect NTFF profiles."""
    import types
    try:
        import antenv.axon_hooks  # noqa: F401
        return True
    except ImportError:
        pass
    try:
        import antenv
        from trn_agent_boot.trn_boot import _ntff_profile_via_ctypes
        state = {"hook": _ntff_profile_via_ctypes('/opt/axon/libaxon_pjrt.so')}
        mod = types.ModuleType("antenv.axon_hooks")
        mod.get_axon_ntff_profile_hook = lambda: state["hook"]
        mod.set_axon_ntff_profile_hook = lambda h: state.__setitem__("hook", h)
        sys.modules["antenv.axon_hooks"] = mod
        antenv.axon_hooks = mod
        return True
    except Exception:
        return False


def _build_nc():
    nc = bacc.Bacc("TRN2", target_bir_lowering=False, debug=False,
                   num_devices=8)

    xt = nc.dram_tensor("xt", [NPAIR, C, L], F32, kind="ExternalInput")
    xtb = nc.dram_tensor("xtb", [NPAIR, C, L], F32, kind="ExternalInput")
    wqkv = nc.dram_tensor("wqkv", [C, 768], F32, kind="ExternalInput")
    wqv = nc.dram_tensor("wqv", [C, 768], F32, kind="ExternalInput")
    thrkv2 = nc.dram_tensor("thrkv2", [128, 1024], F32, kind="ExternalInput")
    thrq = nc.dram_tensor("thrq", [C, 1], F32, kind="ExternalInput")
    sigbq = nc.dram_tensor("sigbq", [C, 1], F32, kind="ExternalInput")
    wps = nc.dram_tensor("wps", [C, C], F32, kind="ExternalInput")
    wpl8 = nc.dram_tensor("wpl8", [128, 512], F8, kind="ExternalInput")
    thrp = nc.dram_tensor("thrp", [C, 1], F32, kind="ExternalInput")
    sigbp = nc.dram_tensor("sigbp", [C, 1], F32, kind="ExternalInput")
    idxrow = nc.dram_tensor("idxrow", [NPAIR, 128, NW * TOPK], I32,
                            kind="ExternalInput")
    out = nc.dram_tensor("out", [NPAIR, C, L], F8, kind="ExternalOutput")
    kvw_dram = nc.dram_tensor("kvw_scratch", [NPAIR * NW * 128, 256], BF16,
                              kind="Internal")

    with TileContext(nc) as tc:
        with (
            tc.tile_pool(name="const", bufs=1) as cpool,
            tc.tile_pool(name="xtp", bufs=1) as xtp,
            tc.tile_pool(name="big", bufs=1) as big,
            tc.tile_pool(name="dbl", bufs=2) as dbl,
            tc.tile_pool(name="fin", bufs=4) as finp,
            tc.tile_pool(name="psW", bufs=2, space="PSUM") as psW,
            tc.tile_pool(name="psN", bufs=2, space="PSUM") as psN,
            tc.tile_pool(name="psB", bufs=2, space="PSUM") as psB,
        ):
            # ---- constants / weights (once) ----
            w_sb = [cpool.tile([128, 768], F32R, tag="wq0", name="wq0"),
                    cpool.tile([128, 768], F32R, tag="wq1", name="wq1")]
            nc.gpsimd.dma_start(w_sb[0][:], wqkv[0:128, :])
            nc.gpsimd.dma_start(w_sb[1][:], wqkv[128:256, :])
            wv_sb = [cpool.tile([128, 768], F32R, tag="wv0", name="wv0"),
                     cpool.tile([128, 768], F32R, tag="wv1", name="wv1")]
            nc.gpsimd.dma_start(wv_sb[0][:], wqv[0:128, :])
            nc.gpsimd.dma_start(wv_sb[1][:], wqv[128:256, :])
            wp_sb = [cpool.tile([128, 256], F32R, tag="wp0", name="wp0"),
                     cpool.tile([128, 256], F32R, tag="wp1", name="wp1")]
            nc.gpsimd.dma_start(wp_sb[0][:], wps[0:128, :])
            nc.gpsimd.dma_start(wp_sb[1][:], wps[128:256, :])
            wpl8_sb = cpool.tile([128, 512], F8, tag="wpl8", name="wpl8")
            nc.gpsimd.dma_start(wpl8_sb[:], wpl8[:])
            thrkv_sb = cpool.tile([128, 1024], F32, tag="thrkv", name="thrkv")
            nc.gpsimd.dma_start(thrkv_sb[:], thrkv2[:])
            thrq_sb = cpool.tile([128, 2], F32, tag="thrq", name="thrq")
            nc.sync.dma_start(thrq_sb[:], thrq.rearrange("(a p) b -> p (a b)", p=128))
            sigbq_sb = cpool.tile([128, 2], F32, tag="sigbq", name="sigbq")
            nc.sync.dma_start(sigbq_sb[:], sigbq.rearrange("(a p) b -> p (a b)", p=128))
            thrp_sb = cpool.tile([128, 2], F32, tag="thrp", name="thrp")
            nc.sync.dma_start(thrp_sb[:], thrp.rearrange("(a p) b -> p (a b)", p=128))
            sigbp_sb = cpool.tile([128, 2], F32, tag="sigbp", name="sigbp")
            nc.sync.dma_start(sigbp_sb[:], sigbp.rearrange("(a p) b -> p (a b)", p=128))
            idx_sb = [cpool.tile([128, NW * TOPK], I32, tag="idx0", name="idx0"),
                      cpool.tile([128, NW * TOPK], I32, tag="idx1", name="idx1")]
            nc.gpsimd.dma_start(idx_sb[0][:], idxrow[0, :, :])
            nc.gpsimd.dma_start(idx_sb[1][:], idxrow[1, :, :])

            # ---- per-pair x quarter tiles (slots shared across pairs) ----
            def load_x(p):
                tiles = []
                for q in range(4):
                    qs = slice(q * 1024, (q + 1) * 1024)
                    tq = {}
                    for nm, src in (("xt0", xt[p, 0:128, qs]),
                                    ("xt1", xt[p, 128:256, qs]),
                                    ("xb0", xtb[p, 0:128, qs]),
                                    ("xb1", xtb[p, 128:256, qs])):
                        t = xtp.tile([128, 1024], F32R, tag=f"{nm}q{q}",
                                     name=f"{nm}q{q}")
                        nc.sync.dma_start(t[:], src.bitcast(F32R))
                        tq[nm] = t
                    tiles.append(tq)
                return tiles

            xq = [None, None]
            kv_sb = [None, None]
            kvw_sb = [None, None]
            kvg_sb = [None, None]
            qt_sb = [None, None]
            outT = [None, None]
            out8 = [None, None]

            def phase_AB(p):
                kv_sb[p] = big.tile([128, 32 * 512], F8, tag="kv", name="kv")
                kvw_sb[p] = dbl.tile([128, NW * 256], BF16, tag="kvw", name="kvw")
                nc.vector.memset(kvw_sb[p][:], 0.0)
                xqp = xq[p]
                for mp in range(16):
                    q = mp // 4
                    ps = psW.tile([128, 1024], F32, tag="psW", name="psW")
                    for h in range(2):
                        m = 2 * mp + h
                        mo = (m % 8) * 128
                        msl = slice(mo, mo + 128)
                        psl = slice(h * 512, (h + 1) * 512)
                        nc.tensor.matmul(ps[:, psl], xqp[q]["xt0"][:, msl],
                                         w_sb[0][:, 256:768], start=True, stop=False,
                                         skip_group_check=True)
                        nc.tensor.matmul(ps[:, psl], xqp[q]["xt0"][:, msl],
                                         wv_sb[0][:, 256:768], start=False, stop=False,
                                         skip_group_check=True)
                        nc.tensor.matmul(ps[:, psl], xqp[q]["xt1"][:, msl],
                                         w_sb[1][:, 256:768], start=False, stop=False,
                                         skip_group_check=True)
                        nc.tensor.matmul(ps[:, psl], xqp[q]["xt1"][:, msl],
                                         wv_sb[1][:, 256:768], start=False, stop=False,
                                         skip_group_check=True)
                        nc.tensor.matmul(ps[:, psl], xqp[q]["xb0"][:, msl],
                                         w_sb[0][:, 256:768], start=False, stop=False,
                                         skip_group_check=True)
                        nc.tensor.matmul(ps[:, psl], xqp[q]["xb1"][:, msl],
                                         w_sb[1][:, 256:768], start=False, stop=True,
                                         skip_group_check=True)
                    nc.vector.tensor_tensor(
                        kv_sb[p][:, mp * 1024:(mp + 1) * 1024],
                        ps[:], thrkv_sb[:], op=GE)
                    # B round after every 4 mp (8 m-tiles): windows 2r, 2r+1
                    if mp % 4 == 3:
                        r = mp // 4
                        kvwf = psB.tile([128, 512], F32, tag="kvwf", name="kvwf")
                        for wl in range(2):
                            j = 2 * r + wl
                            base = j * 2048
                            kvv = kv_sb[p][:, base:base + 1024].rearrange(
                                "p (two x) -> p two x", two=2)
                            kvv2 = kv_sb[p][:, base + 1024:base + 2048].rearrange(
                                "p (two x) -> p two x", two=2)
                            for hp in range(2):
                                ksl = slice(hp * 128, hp * 128 + 128)
                                vsl = slice(256 + hp * 128, 256 + hp * 128 + 128)
                                blk = (2 * wl + hp) * 128
                                nc.tensor.matmul(
                                    kvwf[:, blk:blk + 128],
                                    kvv[:, :, ksl], kvv[:, :, vsl],
                                    start=(wl == 0 and hp == 0), stop=False,
                                    perf_mode=DRMODE, skip_group_check=True)
                                nc.tensor.matmul(
                                    kvwf[:, blk:blk + 128],
                                    kvv2[:, :, ksl], kvv2[:, :, vsl],
                                    start=False, stop=(wl == 1 and hp == 1),
                                    perf_mode=DRMODE, skip_group_check=True)
                        # extract diag [64,64] blocks -> kvw_sb (block-diag, zeros off-diag)
                        for s in range(2):
                            srows = slice(s * 64, (s + 1) * 64)
                            srcap = kvwf[srows, :].rearrange(
                                "q (b e) -> q b e", e=128)[:, :, s * 64:s * 64 + 64]
                            dstap = kvw_sb[p][srows, r * 512:(r + 1) * 512].rearrange(
                                "q (b e) -> q b e", e=128)[:, :, s * 64:s * 64 + 64]
                            if s == 0:
                                nc.vector.tensor_copy(dstap, srcap)
                            else:
                                nc.scalar.copy(dstap, srcap)
                        # write the 2 windows' kvw blocks to DRAM for the gather
                        rows = slice((p * NW + 2 * r) * 128, (p * NW + 2 * r + 2) * 128)
                        nc.sync.dma_start(
                            kvw_dram[rows, :].rearrange("(j p) e -> p j e", p=128),
                            kvw_sb[p][:, r * 512:(r + 1) * 512])

            def phase_qT(p):
                qt_sb[p] = dbl.tile([128, 2 * L], BF16, tag="qt", name="qt")
                xqp = xq[p]
                for g in range(8):
                    q, go = g // 2, (g % 2) * 512
                    gsl = slice(go, go + 512)
                    for dq in range(2):
                        ps = psN.tile([128, 512], F32, tag="psN", name="psQ")
                        dsl = slice(dq * 128, (dq + 1) * 128)
                        nc.tensor.matmul(ps[:], w_sb[0][:, dsl], xqp[q]["xt0"][:, gsl],
                                         start=True, stop=False)
                        nc.tensor.matmul(ps[:], w_sb[0][:, dsl], xqp[q]["xb0"][:, gsl],
                                         start=False, stop=False)
                        nc.tensor.matmul(ps[:], w_sb[1][:, dsl], xqp[q]["xt1"][:, gsl],
                                         start=False, stop=False)
                        nc.tensor.matmul(ps[:], w_sb[1][:, dsl], xqp[q]["xb1"][:, gsl],
                                         start=False, stop=False)
                        nc.tensor.matmul(ps[:], wv_sb[0][:, dsl], xqp[q]["xt0"][:, gsl],
                                         start=False, stop=False)
                        nc.tensor.matmul(ps[:], wv_sb[1][:, dsl], xqp[q]["xt1"][:, gsl],
                                         start=False, stop=True)
                        dst = qt_sb[p][:, dq * L + g * 512: dq * L + (g + 1) * 512]
                        if dq == 0:
                            nc.scalar.activation(dst, ps[:], SIG,
                                                 bias=sigbq_sb[:, 0:1], scale=BIGS)
                        else:
                            nc.vector.tensor_scalar(dst, ps[:], thrq_sb[:, 1:2],
                                                    None, GE)

            def phase_agg(p):
                # 4 gathers per target window: slot 0 initializes, 1-3 accumulate
                kvg_sb[p] = dbl.tile([128, NW * 256], BF16, tag="kvg", name="kvg")
                for n in range(NW):
                    dst = kvg_sb[p][:, n * 256:(n + 1) * 256]
                    for i in range(TOPK):
                        m = n * TOPK + i
                        nc.gpsimd.indirect_dma_start(
                            out=dst, out_offset=None,
                            in_=kvw_dram[:],
                            in_offset=bass.IndirectOffsetOnAxis(
                                ap=idx_sb[p][:, m:m + 1], axis=0),
                            compute_op=(ADD if i > 0 else mybir.AluOpType.bypass),
                        )

            def phase_C(p):
                outT[p] = big.tile([128, 2 * L], F32R, tag="ot", name="ot")
                out8[p] = big.tile([128, 2 * L], F8, tag="o8", name="o8")
                otv = outT[p][:].rearrange("p (two l) -> p two l", two=2)
                o8v = out8[p][:].rearrange("p (two l) -> p two l", two=2)
                for n in range(NW):
                    ps = psW.tile([128, 1024], F32, tag="psW", name="psC")
                    for dq in range(2):
                        nc.tensor.matmul(
                            ps[:, dq * 512:(dq + 1) * 512],
                            kvg_sb[p][:, n * 256 + dq * 128: n * 256 + dq * 128 + 128],
                            qt_sb[p][:, dq * L + n * 512: dq * L + (n + 1) * 512],
                            start=True, stop=True, skip_group_check=True)
                    nsl = slice(n * 512, (n + 1) * 512)
                    nc.vector.tensor_copy(otv[:, :, nsl], ps[:])
                    nc.scalar.activation(o8v[:, :, nsl], ps[:], COPYF, scale=0.03125)

            def phase_D(p):
                w8v = wpl8_sb[:].rearrange("p (two oc) -> p two oc", two=2)
                o8v = out8[p][:].rearrange("p (two x) -> p two x", two=2)
                for g in range(8):
                    gsl = slice(g * 512, (g + 1) * 512)
                    for ct in range(2):
                        ps = psN.tile([128, 512], F32, tag="psN", name="psD")
                        csl = slice(ct * 128, (ct + 1) * 128)
                        nc.tensor.matmul(ps[:], wp_sb[0][:, csl],
                                         outT[p][:, gsl],
                                         start=True, stop=False,
                                         skip_group_check=True)
                        nc.tensor.matmul(ps[:], wp_sb[1][:, csl],
                                         outT[p][:, L + g * 512: L + (g + 1) * 512],
                                         start=False, stop=False,
                                         skip_group_check=True)
                        for sub in range(4):
                            nc.tensor.matmul(
                                ps[:, sub * 128:(sub + 1) * 128],
                                w8v[:, :, csl],
                                o8v[:, :, g * 512 + sub * 128: g * 512 + (sub + 1) * 128],
                                start=False, stop=(sub == 3),
                                perf_mode=DRMODE, skip_group_check=True)
                        fin = finp.tile([128, 512], F8, tag="fin", name="fin")
                        if ct == 0:
                            nc.scalar.activation(fin[:], ps[:], SIG,
                                                 bias=sigbp_sb[:, 0:1], scale=BIGS)
                        else:
                            nc.vector.tensor_scalar(fin[:], ps[:], thrp_sb[:, 1:2],
                                                    None, GE)
                        nc.sync.dma_start(
                            out[p, ct * 128:(ct + 1) * 128, g * 512:(g + 1) * 512],
                            fin[:])

            # ---- zippered emission ----
            xq[0] = load_x(0)
            xq[1] = load_x(1)
            phase_AB(0)
            phase_agg(0)
            phase_qT(0)
            phase_AB(1)
            phase_C(0)
            phase_D(0)
            phase_agg(1)
            phase_qT(1)
            phase_C(1)
            phase_D(1)

    nc.compile()
    return nc


_NC = None


def _f32r_round(a):
    """Round fp32 to the f32r grid (12-bit significand, round-to-nearest)."""
    u = np.ascontiguousarray(a, dtype=np.float32).view(np.uint32)
    u = (u + np.uint32(1 << 11)) & np.uint32(0xFFFFF000)
    return u.view(np.float32)


def kernel(x, W_qkv, b_qkv, W_proj, b_proj):
    global _NC, _EXEC_TIME_NS
    x = np.asarray(x, dtype=np.float32)
    W_qkv = np.asarray(W_qkv, dtype=np.float32)
    b_qkv = np.asarray(b_qkv, dtype=np.float32)
    W_proj = np.asarray(W_proj, dtype=np.float32)
    b_proj = np.asarray(b_proj, dtype=np.float32)

    # ---- host routing: region sums -> attn -> top-k window indices ----
    region = x.sum(axis=0).reshape(B, NW, WIN, C).sum(axis=2)        # [B,NW,C]
    attn_r = np.einsum('bnc,bmc->bnm', region, region)
    idx = np.argsort(-attn_r, axis=-1, kind='stable')[:, :, :TOPK]   # [B,NW,TOPK]

    # ---- common (replicated) inputs ----
    wq_u = _f32r_round(W_qkv)
    wp_u = _f32r_round(W_proj)
    wpl = (W_proj - wp_u) * 16384.0
    wpl8_np = np.empty((128, 512), dtype=ml_dtypes.float8_e4m3)
    wpl8_np[:, 0:256] = wpl[0:128, :].astype(ml_dtypes.float8_e4m3)
    wpl8_np[:, 256:512] = wpl[128:256, :].astype(ml_dtypes.float8_e4m3)
    thrkv1 = 2.0 - b_qkv[None, 256:768]
    common = {
        "wqkv": wq_u,
        "wqv": np.ascontiguousarray(W_qkv - wq_u),
        "thrkv2": np.ascontiguousarray(
            np.broadcast_to(np.tile(thrkv1, (1, 2)), (128, 1024))),
        "thrq": np.ascontiguousarray(2.0 - b_qkv[0:256, None]),
        "sigbq": np.ascontiguousarray(
            -BIGS * (2.0 - b_qkv[0:256, None])).astype(np.float32),
        "wps": np.ascontiguousarray(wp_u * PS),
        "wpl8": wpl8_np,
        "thrp": np.ascontiguousarray(PS * (2.0 - b_proj[:, None])),
        "sigbp": np.ascontiguousarray(
            -BIGS * PS * (2.0 - b_proj[:, None])).astype(np.float32),
    }

    in_maps = []
    pairs = [(t, b) for t in range(T) for b in range(B)]
    for core in range(NCORES):
        mine = pairs[core * NPAIR:(core + 1) * NPAIR]
        xt_full = np.stack([np.ascontiguousarray(x[t, b].T) for (t, b) in mine])
        xt = _f32r_round(xt_full)
        rows = []
        for k, (t, b) in enumerate(mine):
            r = np.empty((128, NW * TOPK), dtype=np.int32)
            for n in range(NW):
                for i in range(TOPK):
                    r[:, n * TOPK + i] = ((k * NW + idx[b, n, i]) * 128
                                          + np.arange(128))
            rows.append(r)
        m = dict(common)
        m["xt"] = xt
        m["xtb"] = xt_full - xt
        m["idxrow"] = np.stack(rows)
        in_maps.append(m)

    if _NC is None:
        _NC = _build_nc()

    traceable = _ensure_ntff_hook()
    try:
        res = bass_utils.run_bass_kernel_spmd(_NC, in_maps,
                                              core_ids=list(range(NCORES)),
                                              trace=traceable)
    except Exception:
        if not traceable:
            raise
        res = bass_utils.run_bass_kernel_spmd(_NC, in_maps,
                                              core_ids=list(range(NCORES)),
                                              trace=False)
    _EXEC_TIME_NS = res.exec_time_ns

    full = np.empty((T, B, L, C), dtype=np.float32)
    for core in range(NCORES):
        mine = pairs[core * NPAIR:(core + 1) * NPAIR]
        o = np.asarray(res.results[core]["out"]).astype(np.float32)  # [NPAIR,C,L]
        for k, (t, b) in enumerate(mine):
            full[t, b] = o[k].T
    return full
